# revision 12
# baseline (speedup 1.0000x reference)
"""Trainium2 Bass kernel for nn_DiscretePolicy (gnn_message_passing).

Reference computation:
  Xn = batchnorm(X)  (training-mode, biased var, eps=1e-5)
  ent = Xn[:, 4:].reshape(B, 100, 2)
  me = leaky_relu(ent @ W_me.T + b_me); me_out = mean_k(me)      # [B, 64]
  h = leaky_relu([Xn[:, :4], me_out] @ W1.T + b1)
  h = leaky_relu(h @ W2.T + b2)
  out = softmax(h @ W3.T + b3)

Strategy (8-way batch-parallel, 2048 rows/core):
  - X is cast-DMA'd to bf16 in a column-permuted layout (balances entity
    pairs across the 4 SBUF partition quadrants), PE-transposed on-chip to
    a feature-major layout XT [128, 2*2048].
  - BatchNorm stats via PE ones-matmuls on the natural-layout tiles; a tiny
    AllReduce combines per-core sums; rsqrt via reciprocal+sqrt+Newton.
    Normalization applied in-place on XT (per-partition scale+shift).
  - leaky_relu(z) is decomposed as alpha*z + (1-alpha)*relu(z). The linear
    part is folded analytically into the first MLP layer; only
    R = sum_k relu(z_k + b_me) is computed at full resolution:
      * entity matmuls: K=32 zero-padded block weights, one entity-pair per
        matmul, 4 concurrent via tile_position row groups, fp32 PSUM
      * relu+bias: split across ScalarE (activation) and VectorE
        (tensor_scalar add+max), 3 PSUM banks per instruction, bf16 out
      * pooling over the 100 entities: PE matmul with a 0/1 selector,
        accumulated in PSUM
  - MLP: leaky layers via max identity — h = a*p + (1-a)*relu(p) computed as
    two accumulating matmuls on (p, relu(p)); softmax via PE transpose to
    batch-major then Exp + reciprocal (no max subtraction: logits are O(1)).
"""

import sys
import numpy as np

sys.path.insert(0, "/opt/trn_rl_repo")

import ml_dtypes

B_FULL, D, H, A = 16384, 204, 64, 32
NCORES = 8
BL = B_FULL // NCORES          # 2048 rows per core
NBT = 4                        # batch tiles per core
NT = BL // NBT                 # 512 columns per batch tile
K_ENT = 100                    # entities
NPAIR = 50                     # entity pairs (2 entities / matmul)
ALPHA = 0.01                   # jax.nn.leaky_relu default negative_slope
EPS = 1e-5
C = 256                        # padded feature columns (bf16 layout)
DPAD = 228                     # host-padded X width (features 204..227 = 0)

# --- column layout: block k of 32 sbuf columns <- features 28k .. 28k+31 ---
# (one uniform overlapping-window DMA per tile; the 4 duplicated columns per
# block are never selected by Wall/msel).  Pair p (features 4+4p..7+4p) lives
# in block k=(4+4p)//28 at column 4+4p+4k; pairs are 4-aligned and blocks
# start at multiples of 28 (also 4-aligned), so pairs never straddle blocks.
PAIR_COL = np.array([4 + 4 * p + 4 * ((4 + 4 * p) // 28) for p in range(NPAIR)])
for p in range(NPAIR):
    c = PAIR_COL[p]
    k = c // 32
    assert c % 4 == 0 and c % 32 <= 24 and 28 * k <= 4 + 4 * p <= 28 * k + 24


def _feat_of_col():
    f = np.full(C, -1, np.int64)
    for c in range(C):
        k, r = c // 32, c % 32
        if r < 28 and 28 * k + r < D:
            f[c] = 28 * k + r
    return f


FEAT_OF_COL = _feat_of_col()

PAIR_FILL = PAIR_COL // 128            # which transpose block (XT region)
PAIR_PART = PAIR_COL % 128             # partition of first row
PAIR_QUAD = PAIR_PART // 32            # row-group quadrant
PAIR_SLOT = (PAIR_PART % 32) // 4      # slot within quadrant (selects lhsT block)

# round-robin issue order across quadrants
_QLISTS = [[p for p in range(NPAIR) if PAIR_QUAD[p] == g] for g in range(4)]
PAIR_ORDER = []
for t in range(max(len(q) for q in _QLISTS)):
    for g in range(4):
        if t < len(_QLISTS[g]):
            PAIR_ORDER.append(_QLISTS[g][t])
assert len(PAIR_ORDER) == NPAIR

_prog_cache = {}


def _build_host_constants(W_me, b_me, W1, b1, W2, b2, W3, b3):
    bf16 = ml_dtypes.bfloat16
    # Wall [128, 8*128]: for quadrant row r (0..31) and slot m: rows 4m..4m+3
    # hold the entity-pair weight block, other rows zero.  Wall same for all
    # quadrants -> replicate pattern to all 128 partitions.
    pat = np.zeros((32, 8 * 128), np.float32)
    for m in range(8):
        for j in range(2):          # entity within pair
            for e in range(2):      # input dim
                # row 4m+2j+e, columns m*128 + (64j .. 64j+63) = W_me[:, e]
                pat[4 * m + 2 * j + e, m * 128 + 64 * j: m * 128 + 64 * (j + 1)] = W_me[:, e]
    Wall = np.tile(pat, (4, 1)).astype(bf16)

    sel = np.zeros((128, 64), np.float32)
    for j in range(2):
        sel[np.arange(64) + 64 * j, np.arange(64)] = 1.0
    selpack = np.concatenate([sel, sel], axis=1).astype(ml_dtypes.float8_e4m3)
    sel = sel.astype(bf16)

    # msel masks: [128, 32] per XT region (cols >=2 all-zero: they produce
    # zero rows 66..95 of pol_vec for free).  Only columns holding a pair
    # contribute: duplicated/pad columns are excluded.
    mselA = np.zeros((128, 32), np.float32)
    mselB = np.zeros((128, 32), np.float32)
    pair_cols = set()
    for p in range(NPAIR):
        for off in range(4):
            pair_cols.add(int(PAIR_COL[p]) + off)
    for c in range(C):
        f = FEAT_OF_COL[c]
        if c in pair_cols and f >= 4:
            (mselA if c < 128 else mselB)[c % 128, (f - 4) % 2] = 1.0
    mselA = mselA.astype(bf16)
    mselB = mselB.astype(bf16)

    bvec = np.tile(b_me, 2).reshape(128, 1).astype(np.float32)

    # first MLP layer folded weights: pol_vec rows 0..63 = R_raw,
    # rows 64..65 = m_raw, rows 96..99 = head (Xn[:, :4]), rest zero.
    W1h = W1[:, :4]
    W1b = W1[:, 4:]
    lhsT_h1 = np.zeros((128, 64), np.float32)
    lhsT_h1[0:64, :] = ((1.0 - ALPHA) / K_ENT) * W1b.T
    lhsT_h1[64:66, :] = (ALPHA / K_ENT) * (W1b @ W_me).T
    lhsT_h1[96:100, :] = W1h.T
    b1vec = (b1 + ALPHA * (W1b @ b_me)).reshape(64, 1).astype(np.float32)

    h2a = (ALPHA * W2).T.astype(np.float32).copy()
    h2b = ((1.0 - ALPHA) * W2).T.astype(np.float32).copy()
    b2vec = b2.reshape(64, 1).astype(np.float32)
    h3a = (ALPHA * W3).T.astype(np.float32).copy()
    h3b = ((1.0 - ALPHA) * W3).T.astype(np.float32).copy()
    b3vec = b3.reshape(32, 1).astype(np.float32)

    ident = np.eye(128, dtype=np.float32).astype(bf16)   # PE transpose identity (bf16)
    ident32 = np.eye(32, dtype=np.float32)               # logits transpose identity (f32)
    onesb = np.ones((128, 1), np.float32).astype(bf16)
    onesf = np.ones((128, 1), np.float32)

    return dict(Wall=Wall, sel=sel, selpack=selpack, mselA=mselA, mselB=mselB, bvec=bvec,
                lhsT_h1=lhsT_h1, b1vec=b1vec, h2a=h2a, h2b=h2b, b2vec=b2vec,
                h3a=h3a, h3b=h3b, b3vec=b3vec, ident=ident, ident32=ident32,
                onesb=onesb, onesf=onesf)


def build_program(num_devices=NCORES):
    """Emit the SPMD Bass program (identical on every core)."""
    from contextlib import ExitStack
    import concourse.bass as bass
    import concourse.bacc as bacc
    import concourse.tile as tile
    from concourse import mybir

    fp32 = mybir.dt.float32
    bf16 = mybir.dt.bfloat16
    fp8 = mybir.dt.float8e4
    ALU = mybir.AluOpType
    ACTF = mybir.ActivationFunctionType

    nc = bacc.Bacc(None, num_devices=num_devices)

    X = nc.declare_dram_parameter("X", [BL, DPAD], fp32, isOutput=False)
    OUT = nc.declare_dram_parameter("OUT", [BL, A], fp32, isOutput=True)
    dparams = {}
    for name, shape, dt in [
        ("Wall", [128, 1024], bf16), ("sel", [128, 64], bf16),
        ("selpack", [128, 128], fp8),
        ("mselA", [128, 32], bf16), ("mselB", [128, 32], bf16),
        ("bvec", [128, 1], fp32), ("lhsT_h1", [128, 64], fp32),
        ("b1vec", [64, 1], fp32), ("h2a", [64, 64], fp32), ("h2b", [64, 64], fp32),
        ("b2vec", [64, 1], fp32), ("h3a", [64, 32], fp32), ("h3b", [64, 32], fp32),
        ("b3vec", [32, 1], fp32), ("ident", [128, 128], bf16),
        ("ident32", [32, 32], fp32), ("onesb", [128, 1], bf16),
        ("onesf", [128, 1], fp32),
    ]:
        dparams[name] = nc.declare_dram_parameter(name, shape, dt, isOutput=False)

    with tile.TileContext(nc) as tc, ExitStack() as ctx:
        singles = ctx.enter_context(tc.tile_pool(name="singles", bufs=1))
        xtp = ctx.enter_context(tc.tile_pool(name="xtp", bufs=1))

        cst = {}
        for name, p in dparams.items():
            t = singles.tile(list(p.shape), p.dtype, tag=f"cst_{name}")
            nc.sync.dma_start(out=t[:], in_=p[:])
            cst[name] = t

        # XT: feature-major bf16, region F at cols F*BL .. F*BL+BL
        xt = xtp.tile([128, 2 * BL], bf16)
        xt3 = xt.rearrange("p (s n) -> p s n", s=2)

        # ---------------- pre-phase: load, stats, transpose ----------------
        with ExitStack() as pre:
            nat = pre.enter_context(tc.tile_pool(name="nat", bufs=6))
            sqp = pre.enter_context(tc.tile_pool(name="sqp", bufs=3))
            pps = pre.enter_context(tc.tile_pool(name="pps", bufs=2, space="PSUM"))
            sps = pre.enter_context(tc.tile_pool(name="sps", bufs=2, space="PSUM"))
            stp = pre.enter_context(tc.tile_pool(name="stp", bufs=4))

            ps_sx = sps.tile([1, C], fp32, tag="psx")
            ps_sq = sps.tile([1, C], fp32, tag="psq")

            ntile = BL // 128
            for i in range(ntile):
                xb = nat.tile([128, C], bf16)
                base = X[128 * i:128 * (i + 1), :]
                src_ap = bass.AP(
                    tensor=base.tensor, offset=base.offset,
                    ap=[[DPAD, 128], [28, 8], [1, 32]],
                )
                nc.gpsimd.dma_start(out=xb[:], in_=src_ap)
                sq = sqp.tile([128, C], fp32)
                nc.vector.tensor_tensor(out=sq[:], in0=xb[:], in1=xb[:], op=ALU.mult)
                nc.tensor.matmul(ps_sx[:], cst["onesb"][:], xb[:],
                                 start=(i == 0), stop=(i == ntile - 1))
                nc.tensor.matmul(ps_sq[:], cst["onesf"][:], sq[:],
                                 start=(i == 0), stop=(i == ntile - 1))
                # transpose both 128-col blocks -> bf16 PSUM -> XT
                pt = pps.tile([128, C], bf16)
                nc.tensor.transpose(pt[:, 0:128], xb[:, 0:128], cst["ident"][:])
                nc.tensor.transpose(pt[:, 128:256], xb[:, 128:256], cst["ident"][:])
                pt3 = pt.rearrange("p (s n) -> p s n", s=2)
                nc.vector.tensor_copy(xt3[:, :, 128 * i:128 * (i + 1)], pt3[:, :, :])

            # --- stats: evacuate sums, AllReduce, reshape per-partition ---
            st_sb = stp.tile([1, 2 * C], fp32)
            nc.scalar.activation(st_sb[:, 0:C], ps_sx[:], ACTF.Copy)
            nc.scalar.activation(st_sb[:, C:2 * C], ps_sq[:], ACTF.Copy)
            cc_in = nc.dram_tensor("cc_in", [1, 2 * C], fp32)
            cc_out = nc.dram_tensor("cc_out", [1, 2 * C], fp32, addr_space="Shared")
            nc.sync.dma_start(out=cc_in[:], in_=st_sb[:])
            nc.gpsimd.collective_compute(
                "AllReduce", ALU.add,
                replica_groups=[list(range(num_devices))],
                ins=[cc_in[:]], outs=[cc_out[:]],
            )
            # view: element (p2, e) = flat[e*2C... e*256... e*C? sums at 0..C-1, sumsq at C..2C-1
            cc_v = cc_out[:].rearrange("a (e p) -> (a p) e", e=2)
            # per-partition stat tiles: statF[p, 0] = sum x, statF[p, 1] = sum x^2
            for F in range(2):
                st = stp.tile([128, 2], fp32, tag=f"st{F}")
                nc.sync.dma_start(out=st[:], in_=cc_v[128 * F:128 * (F + 1), :])
                mu = stp.tile([128, 1], fp32, tag=f"mu{F}")
                nc.vector.tensor_scalar(out=mu[:], in0=st[:, 0:1], scalar1=1.0 / B_FULL,
                                        scalar2=None, op0=ALU.mult)
                ex2 = stp.tile([128, 1], fp32, tag=f"ex2{F}")
                nc.vector.tensor_scalar(out=ex2[:], in0=st[:, 1:2], scalar1=1.0 / B_FULL,
                                        scalar2=None, op0=ALU.mult)
                mu2 = stp.tile([128, 1], fp32, tag=f"mu2{F}")
                nc.vector.tensor_tensor(out=mu2[:], in0=mu[:], in1=mu[:], op=ALU.mult)
                vpe = stp.tile([128, 1], fp32, tag=f"vpe{F}")
                nc.vector.tensor_tensor(out=vpe[:], in0=ex2[:], in1=mu2[:], op=ALU.subtract)
                nc.vector.tensor_scalar(out=vpe[:], in0=vpe[:], scalar1=EPS, scalar2=None,
                                        op0=ALU.add)
                rs = stp.tile([128, 1], fp32, tag=f"rs{F}")
                nc.vector.reciprocal(rs[:], vpe[:])        # rs = 1/(var+eps)
                s0 = stp.tile([128, 1], fp32, tag=f"s0{F}")
                nc.scalar.activation(s0[:], rs[:], ACTF.Sqrt)
                # one Newton step for sqrt: s1 = s0*(1.5 - 0.5*vpe*s0^2)
                # (note vpe = 1/rs)
                u = stp.tile([128, 1], fp32, tag=f"u{F}")
                nc.vector.tensor_tensor(out=u[:], in0=s0[:], in1=s0[:], op=ALU.mult)
                nc.vector.tensor_tensor(out=u[:], in0=u[:], in1=vpe[:], op=ALU.mult)
                nc.vector.tensor_scalar(out=u[:], in0=u[:], scalar1=-0.5, scalar2=1.5,
                                        op0=ALU.mult, op1=ALU.add)
                svec = stp.tile([128, 1], fp32, tag=f"sv{F}")
                nc.vector.tensor_tensor(out=svec[:], in0=s0[:], in1=u[:], op=ALU.mult)
                nmvec = stp.tile([128, 1], fp32, tag=f"nm{F}")
                nc.vector.tensor_tensor(out=nmvec[:], in0=mu[:], in1=svec[:], op=ALU.mult)
                nc.vector.tensor_scalar(out=nmvec[:], in0=nmvec[:], scalar1=-1.0,
                                        scalar2=None, op0=ALU.mult)
                # normalize region F in place: xt = xt*s + (-mu*s)
                nc.vector.tensor_scalar(
                    out=xt[:, BL * F:BL * (F + 1)], in0=xt[:, BL * F:BL * (F + 1)],
                    scalar1=svec[:], scalar2=nmvec[:], op0=ALU.mult, op1=ALU.add)

        # ---------------- main phase ----------------
        zpsp = ctx.enter_context(tc.tile_pool(name="zpsp", bufs=1, space="PSUM"))
        mlpp = ctx.enter_context(tc.tile_pool(name="mlpp", bufs=2, space="PSUM"))
        ypool = ctx.enter_context(tc.tile_pool(name="ypool", bufs=10))
        polp = ctx.enter_context(tc.tile_pool(name="polp", bufs=3))
        mlps = ctx.enter_context(tc.tile_pool(name="mlps", bufs=4))
        outp = ctx.enter_context(tc.tile_pool(name="outp", bufs=4))

        zps = zpsp.tile([128, 6 * 512], fp32)

        for bt in range(NBT):
            col0 = bt * NT
            acc = mlpp.tile([64, NT], fp32, tag="mlp")
            groups = [PAIR_ORDER[i:i + 2] for i in range(0, NPAIR, 2)]
            ngrp = len(groups)

            def emit_z(gi):
                base = (gi * 2) % 6
                for j, p in enumerate(groups[gi]):
                    g = PAIR_QUAD[p]
                    m = PAIR_SLOT[p]
                    F = PAIR_FILL[p]
                    nc.tensor.matmul(
                        zps[:, (base + j) * 512:(base + j + 1) * 512],
                        cst["Wall"][32 * g:32 * (g + 1), 128 * m:128 * (m + 1)],
                        xt[32 * g:32 * (g + 1), BL * F + col0:BL * F + col0 + NT],
                        start=True, stop=True,
                        tile_position=(32 * int(g), 0),
                    )

            ytiles = {}
            selpack3 = cst["selpack"][:].rearrange("p (two f) -> p two f", two=2)

            def emit_relu(gi):
                base = (gi * 2) % 6
                grp = groups[gi]
                n = len(grp)
                y = ypool.tile([128, 2, 512], fp8)
                ytiles[gi] = y
                yf = y.rearrange("p two d -> p (two d)")
                zv = zps[:, base * 512:(base + n) * 512]
                if gi % 2 == 0:
                    nc.scalar.activation(yf[:, 0:n * 512], zv, ACTF.Relu,
                                         bias=cst["bvec"][:], scale=1.0)
                else:
                    nc.vector.tensor_scalar(out=yf[:, 0:n * 512], in0=zv,
                                            scalar1=cst["bvec"][:], scalar2=0.0,
                                            op0=ALU.add, op1=ALU.max)

            def emit_pool(gi):
                y = ytiles.pop(gi)
                nc.tensor.matmul(acc[:], selpack3, y[:, :, :],
                                 start=(gi == 0), stop=(gi == ngrp - 1),
                                 perf_mode=mybir.MatmulPerfMode.DoubleRow)

            # software pipeline: z-matmuls 3 groups ahead of relu (6 PSUM
            # banks = 3 groups of 2 in flight; relu(gi-3) must precede z(gi)
            # in program order since they share slices), pool lags 3 more.
            for gi in range(ngrp + 6):
                if gi >= 6:
                    emit_pool(gi - 6)
                if 3 <= gi < ngrp + 3:
                    emit_relu(gi - 3)
                if gi < ngrp:
                    emit_z(gi)

            # ---- MLP tail ----
            # pol rows: 0..63 = R_raw, 64..65 = m_raw (66..95 zero via msel
            # zero columns), 96..99 = head (100..127 junk; lhsT_h1 rows are 0)
            pol = polp.tile([128, NT], fp32)
            nc.scalar.activation(pol[0:64, :], acc[:], ACTF.Copy)       # R_raw
            ps_m = mlpp.tile([32, NT], fp32, tag="mlp")
            nc.tensor.matmul(ps_m[:], cst["mselA"][:], xt[:, col0:col0 + NT],
                             start=True, stop=False)
            nc.tensor.matmul(ps_m[:], cst["mselB"][:], xt[:, BL + col0:BL + col0 + NT],
                             start=False, stop=True)
            nc.scalar.activation(pol[64:96, :], ps_m[:], ACTF.Copy)     # m_raw + zeros
            nc.scalar.activation(pol[96:128, :], xt[0:32, col0:col0 + NT], ACTF.Copy)

            ps_h1 = mlpp.tile([64, NT], fp32, tag="mlp")
            nc.tensor.matmul(ps_h1[:], cst["lhsT_h1"][:], pol[:], start=True, stop=True)
            p1 = mlps.tile([64, NT], fp32, tag="p")
            r1 = mlps.tile([64, NT], fp32, tag="r")
            nc.vector.tensor_scalar(out=p1[:], in0=ps_h1[:], scalar1=cst["b1vec"][:],
                                    scalar2=None, op0=ALU.add)
            nc.scalar.activation(r1[:], ps_h1[:], ACTF.Relu, bias=cst["b1vec"][:])

            ps_h2 = mlpp.tile([64, NT], fp32, tag="mlp")
            nc.tensor.matmul(ps_h2[:], cst["h2a"][:], p1[:], start=True, stop=False)
            nc.tensor.matmul(ps_h2[:], cst["h2b"][:], r1[:], start=False, stop=True)
            p2 = mlps.tile([64, NT], fp32, tag="p")
            r2 = mlps.tile([64, NT], fp32, tag="r")
            nc.vector.tensor_scalar(out=p2[:], in0=ps_h2[:], scalar1=cst["b2vec"][:],
                                    scalar2=None, op0=ALU.add)
            nc.scalar.activation(r2[:], ps_h2[:], ACTF.Relu, bias=cst["b2vec"][:])

            ps_lg = mlpp.tile([32, NT], fp32, tag="mlp")
            nc.tensor.matmul(ps_lg[:], cst["h3a"][:], p2[:], start=True, stop=False)
            nc.tensor.matmul(ps_lg[:], cst["h3b"][:], r2[:], start=False, stop=True)
            lg = mlps.tile([32, NT], fp32, tag="lg")
            nc.scalar.activation(lg[:], ps_lg[:], ACTF.Identity, bias=cst["b3vec"][:])

            # ---- softmax over A=32 (transpose to batch-major) ----
            ps_tr = mlpp.tile([128, 128], fp32, tag="mlp")
            for s in range(4):
                nc.tensor.transpose(ps_tr[:, 32 * s:32 * (s + 1)],
                                    lg[:, 128 * s:128 * (s + 1)], cst["ident32"][:])
            esb = outp.tile([128, 128], fp32, tag="e")
            nc.scalar.activation(esb[:], ps_tr[:], ACTF.Exp)
            e3 = esb.rearrange("p (s a) -> p s a", s=4)
            sums = outp.tile([128, 4], fp32, tag="sums")
            nc.vector.tensor_reduce(out=sums[:], in_=e3[:, :, :],
                                    axis=mybir.AxisListType.X, op=ALU.add)
            rec = outp.tile([128, 4], fp32, tag="rec")
            nc.vector.reciprocal(rec[:], sums[:])
            fin = outp.tile([128, 128], fp32, tag="fin")
            fin3 = fin.rearrange("p (s a) -> p s a", s=4)
            rec_b = rec[:].unsqueeze(2).broadcast_to([128, 4, 32])
            nc.vector.tensor_tensor(out=fin3[:, :, :], in0=e3[:, :, :], in1=rec_b,
                                    op=ALU.mult)
            for s in range(4):
                nc.sync.dma_start(
                    out=OUT[col0 + 128 * s: col0 + 128 * (s + 1), :],
                    in_=fin3[:, s, :],
                )
    nc.finalize()
    return nc


def kernel(**inputs):
    X = np.asarray(inputs["X"], np.float32)
    consts = _build_host_constants(
        np.asarray(inputs["W_me"], np.float32), np.asarray(inputs["b_me"], np.float32),
        np.asarray(inputs["W1"], np.float32), np.asarray(inputs["b1"], np.float32),
        np.asarray(inputs["W2"], np.float32), np.asarray(inputs["b2"], np.float32),
        np.asarray(inputs["W3"], np.float32), np.asarray(inputs["b3"], np.float32),
    )
    from concourse.bass_utils import run_bass_kernel_spmd

    if "nc" not in _prog_cache:
        _prog_cache["nc"] = build_program(NCORES)
    nc = _prog_cache["nc"]

    Xpad = np.zeros((B_FULL, DPAD), np.float32)
    Xpad[:, :D] = X
    in_maps = []
    for i in range(NCORES):
        m = {"X": np.ascontiguousarray(Xpad[i * BL:(i + 1) * BL])}
        m.update(consts)
        in_maps.append(m)
    res = run_bass_kernel_spmd(nc, in_maps, list(range(NCORES)))
    out = np.concatenate([res.results[i]["OUT"] for i in range(NCORES)], axis=0)
    return out.astype(np.float32)


# revision 14
# speedup vs baseline: 1.1048x; 1.1048x over previous
"""Trainium2 Bass kernel for nn_DiscretePolicy (gnn_message_passing).

Reference computation:
  Xn = batchnorm(X)  (training-mode, biased var, eps=1e-5)
  ent = Xn[:, 4:].reshape(B, 100, 2)
  me = leaky_relu(ent @ W_me.T + b_me); me_out = mean_k(me)      # [B, 64]
  h = leaky_relu([Xn[:, :4], me_out] @ W1.T + b1)
  h = leaky_relu(h @ W2.T + b2)
  out = softmax(h @ W3.T + b3)

Strategy (8-way batch-parallel, 2048 rows/core):
  - X is cast-DMA'd to bf16 in a column-permuted layout (balances entity
    pairs across the 4 SBUF partition quadrants), PE-transposed on-chip to
    a feature-major layout XT [128, 2*2048].
  - BatchNorm stats via PE ones-matmuls on the natural-layout tiles; a tiny
    AllReduce combines per-core sums; rsqrt via reciprocal+sqrt+Newton.
    Normalization applied in-place on XT (per-partition scale+shift).
  - leaky_relu(z) is decomposed as alpha*z + (1-alpha)*relu(z). The linear
    part is folded analytically into the first MLP layer; only
    R = sum_k relu(z_k + b_me) is computed at full resolution:
      * entity matmuls: K=32 zero-padded block weights, one entity-pair per
        matmul, 4 concurrent via tile_position row groups, fp32 PSUM
      * relu+bias: split across ScalarE (activation) and VectorE
        (tensor_scalar add+max), 3 PSUM banks per instruction, bf16 out
      * pooling over the 100 entities: PE matmul with a 0/1 selector,
        accumulated in PSUM
  - MLP: leaky layers via max identity — h = a*p + (1-a)*relu(p) computed as
    two accumulating matmuls on (p, relu(p)); softmax via PE transpose to
    batch-major then Exp + reciprocal (no max subtraction: logits are O(1)).
"""

import sys
import numpy as np

sys.path.insert(0, "/opt/trn_rl_repo")

import ml_dtypes

B_FULL, D, H, A = 16384, 204, 64, 32
NCORES = 8
BL = B_FULL // NCORES          # 2048 rows per core
NBT = 4                        # batch tiles per core
NT = BL // NBT                 # 512 columns per batch tile
K_ENT = 100                    # entities
NPAIR = 50                     # entity pairs (2 entities / matmul)
ALPHA = 0.01                   # jax.nn.leaky_relu default negative_slope
EPS = 1e-5
C = 256                        # padded feature columns (bf16 layout)
DPAD = 228                     # host-padded X width (features 204..227 = 0)

# --- column layout: block k of 32 sbuf columns <- features 28k .. 28k+31 ---
# (one uniform overlapping-window DMA per tile; the 4 duplicated columns per
# block are never selected by Wall/msel).  Pair p (features 4+4p..7+4p) lives
# in block k=(4+4p)//28 at column 4+4p+4k; pairs are 4-aligned and blocks
# start at multiples of 28 (also 4-aligned), so pairs never straddle blocks.
PAIR_COL = np.array([4 + 4 * p + 4 * ((4 + 4 * p) // 28) for p in range(NPAIR)])
for p in range(NPAIR):
    c = PAIR_COL[p]
    k = c // 32
    assert c % 4 == 0 and c % 32 <= 24 and 28 * k <= 4 + 4 * p <= 28 * k + 24


def _feat_of_col():
    f = np.full(C, -1, np.int64)
    for c in range(C):
        k, r = c // 32, c % 32
        if r < 28 and 28 * k + r < D:
            f[c] = 28 * k + r
    return f


FEAT_OF_COL = _feat_of_col()

PAIR_FILL = PAIR_COL // 128            # which transpose block (XT region)
PAIR_PART = PAIR_COL % 128             # partition of first row
PAIR_QUAD = PAIR_PART // 32            # row-group quadrant
PAIR_SLOT = (PAIR_PART % 32) // 4      # slot within quadrant (selects lhsT block)

# round-robin issue order across quadrants
_QLISTS = [[p for p in range(NPAIR) if PAIR_QUAD[p] == g] for g in range(4)]
PAIR_ORDER = []
for t in range(max(len(q) for q in _QLISTS)):
    for g in range(4):
        if t < len(_QLISTS[g]):
            PAIR_ORDER.append(_QLISTS[g][t])
assert len(PAIR_ORDER) == NPAIR

_prog_cache = {}


def _build_host_constants(W_me, b_me, W1, b1, W2, b2, W3, b3):
    bf16 = ml_dtypes.bfloat16
    # Wall [128, 8*128]: for quadrant row r (0..31) and slot m: rows 4m..4m+3
    # hold the entity-pair weight block, other rows zero.  Wall same for all
    # quadrants -> replicate pattern to all 128 partitions.
    pat = np.zeros((32, 8 * 128), np.float32)
    for m in range(8):
        for j in range(2):          # entity within pair
            for e in range(2):      # input dim
                # row 4m+2j+e, columns m*128 + (64j .. 64j+63) = W_me[:, e]
                pat[4 * m + 2 * j + e, m * 128 + 64 * j: m * 128 + 64 * (j + 1)] = W_me[:, e]
    Wall = np.tile(pat, (4, 1)).astype(bf16)

    sel = np.zeros((128, 64), np.float32)
    for j in range(2):
        sel[np.arange(64) + 64 * j, np.arange(64)] = 1.0
    selpack = np.concatenate([sel, sel], axis=1).astype(ml_dtypes.float8_e4m3)
    sel = sel.astype(bf16)

    # msel masks: [128, 32] per XT region (cols >=2 all-zero: they produce
    # zero rows 66..95 of pol_vec for free).  Only columns holding a pair
    # contribute: duplicated/pad columns are excluded.
    mselA = np.zeros((128, 32), np.float32)
    mselB = np.zeros((128, 32), np.float32)
    pair_cols = set()
    for p in range(NPAIR):
        for off in range(4):
            pair_cols.add(int(PAIR_COL[p]) + off)
    for c in range(C):
        f = FEAT_OF_COL[c]
        if c in pair_cols and f >= 4:
            (mselA if c < 128 else mselB)[c % 128, (f - 4) % 2] = 1.0
    mselA = mselA.astype(bf16)
    mselB = mselB.astype(bf16)

    bvec = np.tile(b_me, 2).reshape(128, 1).astype(np.float32)

    # first MLP layer folded weights: pol_vec rows 0..63 = R_raw,
    # rows 64..65 = m_raw, rows 96..99 = head (Xn[:, :4]), rest zero.
    W1h = W1[:, :4]
    W1b = W1[:, 4:]
    lhsT_h1 = np.zeros((128, 64), np.float32)
    lhsT_h1[0:64, :] = ((1.0 - ALPHA) / K_ENT) * W1b.T
    lhsT_h1[64:66, :] = (ALPHA / K_ENT) * (W1b @ W_me).T
    lhsT_h1[96:100, :] = W1h.T
    b1vec = (b1 + ALPHA * (W1b @ b_me)).reshape(64, 1).astype(np.float32)

    h2a = (ALPHA * W2).T.astype(np.float32).copy()
    h2b = ((1.0 - ALPHA) * W2).T.astype(np.float32).copy()
    b2vec = b2.reshape(64, 1).astype(np.float32)
    h3a = (ALPHA * W3).T.astype(np.float32).copy()
    h3b = ((1.0 - ALPHA) * W3).T.astype(np.float32).copy()
    b3vec = b3.reshape(32, 1).astype(np.float32)

    ident = np.eye(128, dtype=np.float32).astype(bf16)   # PE transpose identity (bf16)
    ident32 = np.eye(32, dtype=np.float32)               # logits transpose identity (f32)
    onesb = np.ones((128, 1), np.float32).astype(bf16)
    onesf = np.ones((128, 1), np.float32)

    return dict(Wall=Wall, sel=sel, selpack=selpack, mselA=mselA, mselB=mselB, bvec=bvec,
                lhsT_h1=lhsT_h1, b1vec=b1vec, h2a=h2a, h2b=h2b, b2vec=b2vec,
                h3a=h3a, h3b=h3b, b3vec=b3vec, ident=ident, ident32=ident32,
                onesb=onesb, onesf=onesf)


def build_program(num_devices=NCORES):
    """Emit the SPMD Bass program (identical on every core)."""
    from contextlib import ExitStack
    import concourse.bass as bass
    import concourse.bacc as bacc
    import concourse.tile as tile
    from concourse import mybir

    fp32 = mybir.dt.float32
    bf16 = mybir.dt.bfloat16
    fp8 = mybir.dt.float8e4
    ALU = mybir.AluOpType
    ACTF = mybir.ActivationFunctionType

    nc = bacc.Bacc(None, num_devices=num_devices)

    X = nc.declare_dram_parameter("X", [BL, DPAD], fp32, isOutput=False)
    OUT = nc.declare_dram_parameter("OUT", [BL, A], fp32, isOutput=True)
    dparams = {}
    for name, shape, dt in [
        ("Wall", [128, 1024], bf16), ("sel", [128, 64], bf16),
        ("selpack", [128, 128], fp8),
        ("mselA", [128, 32], bf16), ("mselB", [128, 32], bf16),
        ("bvec", [128, 1], fp32), ("lhsT_h1", [128, 64], fp32),
        ("b1vec", [64, 1], fp32), ("h2a", [64, 64], fp32), ("h2b", [64, 64], fp32),
        ("b2vec", [64, 1], fp32), ("h3a", [64, 32], fp32), ("h3b", [64, 32], fp32),
        ("b3vec", [32, 1], fp32), ("ident", [128, 128], bf16),
        ("ident32", [32, 32], fp32), ("onesb", [128, 1], bf16),
        ("onesf", [128, 1], fp32),
    ]:
        dparams[name] = nc.declare_dram_parameter(name, shape, dt, isOutput=False)

    with tile.TileContext(nc) as tc, ExitStack() as ctx:
        singles = ctx.enter_context(tc.tile_pool(name="singles", bufs=1))
        xtp = ctx.enter_context(tc.tile_pool(name="xtp", bufs=1))

        cst = {}
        for name, p in dparams.items():
            t = singles.tile(list(p.shape), p.dtype, tag=f"cst_{name}")
            nc.sync.dma_start(out=t[:], in_=p[:])
            cst[name] = t

        # XT: feature-major bf16, region F at cols F*BL .. F*BL+BL
        xt = xtp.tile([128, 2 * BL], bf16)
        xt3 = xt.rearrange("p (s n) -> p s n", s=2)

        # ---------------- pre-phase: load, stats, transpose ----------------
        with ExitStack() as pre:
            nat = pre.enter_context(tc.tile_pool(name="nat", bufs=6))
            f32p = pre.enter_context(tc.tile_pool(name="f32p", bufs=4))
            sqp = pre.enter_context(tc.tile_pool(name="sqp", bufs=3))
            pps = pre.enter_context(tc.tile_pool(name="pps", bufs=2, space="PSUM"))
            sps = pre.enter_context(tc.tile_pool(name="sps", bufs=2, space="PSUM"))
            stp = pre.enter_context(tc.tile_pool(name="stp", bufs=4))

            ps_sx = sps.tile([1, C], fp32, tag="psx")
            ps_sq = sps.tile([1, C], fp32, tag="psq")

            ntile = BL // 128
            for i in range(ntile):
                xf = f32p.tile([128, DPAD], fp32)
                nc.sync.dma_start(out=xf[:], in_=X[128 * i:128 * (i + 1), :])
                # cast + overlapping-window expand: block k of 32 cols <-
                # features 28k..28k+31 (f32 -> bf16)
                xb = nat.tile([128, C], bf16)
                win = bass.AP(
                    tensor=xf.tensor, offset=xf.offset,
                    ap=[list(xf.ap[0]), [28, 8], [1, 32]],
                )
                nc.vector.tensor_copy(xb[:], win)
                sq = sqp.tile([128, C], fp32)
                nc.vector.tensor_tensor(out=sq[:], in0=xb[:], in1=xb[:], op=ALU.mult)
                nc.tensor.matmul(ps_sx[:], cst["onesb"][:], xb[:],
                                 start=(i == 0), stop=(i == ntile - 1))
                nc.tensor.matmul(ps_sq[:], cst["onesf"][:], sq[:],
                                 start=(i == 0), stop=(i == ntile - 1))
                # transpose both 128-col blocks -> bf16 PSUM -> XT
                pt = pps.tile([128, C], bf16)
                nc.tensor.transpose(pt[:, 0:128], xb[:, 0:128], cst["ident"][:])
                nc.tensor.transpose(pt[:, 128:256], xb[:, 128:256], cst["ident"][:])
                pt3 = pt.rearrange("p (s n) -> p s n", s=2)
                nc.vector.tensor_copy(xt3[:, :, 128 * i:128 * (i + 1)], pt3[:, :, :])

            # --- stats: evacuate sums, AllReduce, reshape per-partition ---
            st_sb = stp.tile([1, 2 * C], fp32)
            nc.scalar.activation(st_sb[:, 0:C], ps_sx[:], ACTF.Copy)
            nc.scalar.activation(st_sb[:, C:2 * C], ps_sq[:], ACTF.Copy)
            cc_in = nc.dram_tensor("cc_in", [1, 2 * C], fp32)
            cc_out = nc.dram_tensor("cc_out", [1, 2 * C], fp32, addr_space="Shared")
            nc.sync.dma_start(out=cc_in[:], in_=st_sb[:])
            nc.gpsimd.collective_compute(
                "AllReduce", ALU.add,
                replica_groups=[list(range(num_devices))],
                ins=[cc_in[:]], outs=[cc_out[:]],
            )
            # view: element (p2, e) = flat[e*2C... e*256... e*C? sums at 0..C-1, sumsq at C..2C-1
            cc_v = cc_out[:].rearrange("a (e p) -> (a p) e", e=2)
            # per-partition stats for both regions at once: st[p, F] layout
            st = stp.tile([128, 2, 2], fp32, tag="st")   # [p, F, (sx, sq)]
            for F in range(2):
                nc.sync.dma_start(out=st[:, F, :], in_=cc_v[128 * F:128 * (F + 1), :])
            stf = st.rearrange("p f e -> p (f e)")
            mus = stp.tile([128, 2], fp32, tag="mus")    # mu per region
            ex2 = stp.tile([128, 2], fp32, tag="ex2")
            nc.vector.tensor_scalar(out=mus[:], in0=stf[:, 0:4:2], scalar1=1.0 / B_FULL,
                                    scalar2=None, op0=ALU.mult)
            nc.vector.tensor_scalar(out=ex2[:], in0=stf[:, 1:4:2], scalar1=1.0 / B_FULL,
                                    scalar2=None, op0=ALU.mult)
            mu2 = stp.tile([128, 2], fp32, tag="mu2")
            nc.vector.tensor_tensor(out=mu2[:], in0=mus[:], in1=mus[:], op=ALU.mult)
            vpe = stp.tile([128, 2], fp32, tag="vpe")
            nc.vector.tensor_tensor(out=vpe[:], in0=ex2[:], in1=mu2[:], op=ALU.subtract)
            nc.vector.tensor_scalar(out=vpe[:], in0=vpe[:], scalar1=EPS, scalar2=None,
                                    op0=ALU.add)
            rs = stp.tile([128, 2], fp32, tag="rs")
            nc.vector.reciprocal(rs[:], vpe[:])          # 1/(var+eps)
            s0 = stp.tile([128, 2], fp32, tag="s0")
            nc.scalar.activation(s0[:], rs[:], ACTF.Sqrt)
            # one Newton step for sqrt: s = s0*(1.5 - 0.5*vpe*s0^2); vpe = 1/rs
            u = stp.tile([128, 2], fp32, tag="u")
            nc.vector.tensor_tensor(out=u[:], in0=s0[:], in1=s0[:], op=ALU.mult)
            nc.vector.tensor_tensor(out=u[:], in0=u[:], in1=vpe[:], op=ALU.mult)
            nc.vector.tensor_scalar(out=u[:], in0=u[:], scalar1=-0.5, scalar2=1.5,
                                    op0=ALU.mult, op1=ALU.add)
            svec = stp.tile([128, 2], fp32, tag="sv")
            nc.vector.tensor_tensor(out=svec[:], in0=s0[:], in1=u[:], op=ALU.mult)
            nmvec = stp.tile([128, 2], fp32, tag="nm")
            nc.vector.tensor_tensor(out=nmvec[:], in0=mus[:], in1=svec[:], op=ALU.mult)
            nc.vector.tensor_scalar(out=nmvec[:], in0=nmvec[:], scalar1=-1.0,
                                    scalar2=None, op0=ALU.mult)
            for F in range(2):
                nc.vector.tensor_scalar(
                    out=xt[:, BL * F:BL * (F + 1)], in0=xt[:, BL * F:BL * (F + 1)],
                    scalar1=svec[:, F:F + 1], scalar2=nmvec[:, F:F + 1],
                    op0=ALU.mult, op1=ALU.add)

        # ---------------- main phase ----------------
        zpsp = ctx.enter_context(tc.tile_pool(name="zpsp", bufs=1, space="PSUM"))
        mlpp = ctx.enter_context(tc.tile_pool(name="mlpp", bufs=2, space="PSUM"))
        ypool = ctx.enter_context(tc.tile_pool(name="ypool", bufs=10))
        polp = ctx.enter_context(tc.tile_pool(name="polp", bufs=3))
        mlps = ctx.enter_context(tc.tile_pool(name="mlps", bufs=4))
        outp = ctx.enter_context(tc.tile_pool(name="outp", bufs=4))

        zps = zpsp.tile([128, 6 * 512], fp32)

        for bt in range(NBT):
            col0 = bt * NT
            acc = mlpp.tile([64, NT], fp32, tag="mlp")
            groups = [PAIR_ORDER[i:i + 3] for i in range(0, NPAIR, 3)]
            ngrp = len(groups)

            def emit_z(gi):
                base = (gi * 3) % 6
                for j, p in enumerate(groups[gi]):
                    g = PAIR_QUAD[p]
                    m = PAIR_SLOT[p]
                    F = PAIR_FILL[p]
                    nc.tensor.matmul(
                        zps[:, (base + j) * 512:(base + j + 1) * 512],
                        cst["Wall"][32 * g:32 * (g + 1), 128 * m:128 * (m + 1)],
                        xt[32 * g:32 * (g + 1), BL * F + col0:BL * F + col0 + NT],
                        start=True, stop=True,
                        tile_position=(32 * int(g), 0),
                    )

            ytiles = {}

            def emit_relu(gi):
                base = (gi * 3) % 6
                grp = groups[gi]
                n = len(grp)
                y = ypool.tile([128, 3 * 512], bf16)
                ytiles[gi] = y
                zv = zps[:, base * 512:(base + n) * 512]
                if gi % 2 == 0:
                    nc.scalar.activation(y[:, 0:n * 512], zv, ACTF.Relu,
                                         bias=cst["bvec"][:], scale=1.0)
                else:
                    nc.vector.tensor_scalar(out=y[:, 0:n * 512], in0=zv,
                                            scalar1=cst["bvec"][:], scalar2=0.0,
                                            op0=ALU.add, op1=ALU.max)

            def emit_pool(gi):
                y = ytiles.pop(gi)
                for j, p in enumerate(groups[gi]):
                    idx = gi * 3 + j
                    nc.tensor.matmul(acc[:], cst["sel"][:], y[:, j * 512:(j + 1) * 512],
                                     start=(idx == 0), stop=(idx == NPAIR - 1))

            # software pipeline: z-matmuls 2 groups ahead of relu (6 PSUM
            # banks = 2 groups of 3 in flight; relu(gi-2) must precede z(gi)
            # in program order since they share slices), pool lags 2 more.
            for gi in range(ngrp + 4):
                if gi >= 4:
                    emit_pool(gi - 4)
                if 2 <= gi < ngrp + 2:
                    emit_relu(gi - 2)
                if gi < ngrp:
                    emit_z(gi)

            # ---- MLP tail ----
            # pol rows: 0..63 = R_raw, 64..65 = m_raw (66..95 zero via msel
            # zero columns), 96..99 = head (100..127 junk; lhsT_h1 rows are 0)
            pol = polp.tile([128, NT], fp32)
            nc.scalar.activation(pol[0:64, :], acc[:], ACTF.Copy)       # R_raw
            ps_m = mlpp.tile([32, NT], fp32, tag="mlp")
            nc.tensor.matmul(ps_m[:], cst["mselA"][:], xt[:, col0:col0 + NT],
                             start=True, stop=False)
            nc.tensor.matmul(ps_m[:], cst["mselB"][:], xt[:, BL + col0:BL + col0 + NT],
                             start=False, stop=True)
            nc.scalar.activation(pol[64:96, :], ps_m[:], ACTF.Copy)     # m_raw + zeros
            nc.scalar.activation(pol[96:128, :], xt[0:32, col0:col0 + NT], ACTF.Copy)

            ps_h1 = mlpp.tile([64, NT], fp32, tag="mlp")
            nc.tensor.matmul(ps_h1[:], cst["lhsT_h1"][:], pol[:], start=True, stop=True)
            p1 = mlps.tile([64, NT], fp32, tag="p")
            r1 = mlps.tile([64, NT], fp32, tag="r")
            nc.vector.tensor_scalar(out=p1[:], in0=ps_h1[:], scalar1=cst["b1vec"][:],
                                    scalar2=None, op0=ALU.add)
            nc.scalar.activation(r1[:], ps_h1[:], ACTF.Relu, bias=cst["b1vec"][:])

            ps_h2 = mlpp.tile([64, NT], fp32, tag="mlp")
            nc.tensor.matmul(ps_h2[:], cst["h2a"][:], p1[:], start=True, stop=False)
            nc.tensor.matmul(ps_h2[:], cst["h2b"][:], r1[:], start=False, stop=True)
            p2 = mlps.tile([64, NT], fp32, tag="p")
            r2 = mlps.tile([64, NT], fp32, tag="r")
            nc.vector.tensor_scalar(out=p2[:], in0=ps_h2[:], scalar1=cst["b2vec"][:],
                                    scalar2=None, op0=ALU.add)
            nc.scalar.activation(r2[:], ps_h2[:], ACTF.Relu, bias=cst["b2vec"][:])

            ps_lg = mlpp.tile([32, NT], fp32, tag="mlp")
            nc.tensor.matmul(ps_lg[:], cst["h3a"][:], p2[:], start=True, stop=False)
            nc.tensor.matmul(ps_lg[:], cst["h3b"][:], r2[:], start=False, stop=True)
            lg = mlps.tile([32, NT], fp32, tag="lg")
            nc.scalar.activation(lg[:], ps_lg[:], ACTF.Identity, bias=cst["b3vec"][:])

            # ---- softmax over A=32 (transpose to batch-major) ----
            ps_tr = mlpp.tile([128, 128], fp32, tag="mlp")
            for s in range(4):
                nc.tensor.transpose(ps_tr[:, 32 * s:32 * (s + 1)],
                                    lg[:, 128 * s:128 * (s + 1)], cst["ident32"][:])
            esb = outp.tile([128, 128], fp32, tag="e")
            nc.scalar.activation(esb[:], ps_tr[:], ACTF.Exp)
            e3 = esb.rearrange("p (s a) -> p s a", s=4)
            sums = outp.tile([128, 4], fp32, tag="sums")
            nc.vector.tensor_reduce(out=sums[:], in_=e3[:, :, :],
                                    axis=mybir.AxisListType.X, op=ALU.add)
            rec = outp.tile([128, 4], fp32, tag="rec")
            nc.vector.reciprocal(rec[:], sums[:])
            fin = outp.tile([128, 128], fp32, tag="fin")
            fin3 = fin.rearrange("p (s a) -> p s a", s=4)
            rec_b = rec[:].unsqueeze(2).broadcast_to([128, 4, 32])
            nc.vector.tensor_tensor(out=fin3[:, :, :], in0=e3[:, :, :], in1=rec_b,
                                    op=ALU.mult)
            for s in range(4):
                nc.sync.dma_start(
                    out=OUT[col0 + 128 * s: col0 + 128 * (s + 1), :],
                    in_=fin3[:, s, :],
                )
    nc.finalize()
    return nc


def kernel(**inputs):
    X = np.asarray(inputs["X"], np.float32)
    consts = _build_host_constants(
        np.asarray(inputs["W_me"], np.float32), np.asarray(inputs["b_me"], np.float32),
        np.asarray(inputs["W1"], np.float32), np.asarray(inputs["b1"], np.float32),
        np.asarray(inputs["W2"], np.float32), np.asarray(inputs["b2"], np.float32),
        np.asarray(inputs["W3"], np.float32), np.asarray(inputs["b3"], np.float32),
    )
    from concourse.bass_utils import run_bass_kernel_spmd

    if "nc" not in _prog_cache:
        _prog_cache["nc"] = build_program(NCORES)
    nc = _prog_cache["nc"]

    Xpad = np.zeros((B_FULL, DPAD), np.float32)
    Xpad[:, :D] = X
    in_maps = []
    for i in range(NCORES):
        m = {"X": np.ascontiguousarray(Xpad[i * BL:(i + 1) * BL])}
        m.update(consts)
        in_maps.append(m)
    res = run_bass_kernel_spmd(nc, in_maps, list(range(NCORES)))
    out = np.concatenate([res.results[i]["OUT"] for i in range(NCORES)], axis=0)
    return out.astype(np.float32)


# revision 17
# speedup vs baseline: 1.1187x; 1.0126x over previous
"""Trainium2 Bass kernel for nn_DiscretePolicy (gnn_message_passing).

Reference computation:
  Xn = batchnorm(X)  (training-mode, biased var, eps=1e-5)
  ent = Xn[:, 4:].reshape(B, 100, 2)
  me = leaky_relu(ent @ W_me.T + b_me); me_out = mean_k(me)      # [B, 64]
  h = leaky_relu([Xn[:, :4], me_out] @ W1.T + b1)
  h = leaky_relu(h @ W2.T + b2)
  out = softmax(h @ W3.T + b3)

Strategy (8-way batch-parallel, 2048 rows/core):
  - X is cast-DMA'd to bf16 in a column-permuted layout (balances entity
    pairs across the 4 SBUF partition quadrants), PE-transposed on-chip to
    a feature-major layout XT [128, 2*2048].
  - BatchNorm stats via PE ones-matmuls on the natural-layout tiles; a tiny
    AllReduce combines per-core sums; rsqrt via reciprocal+sqrt+Newton.
    Normalization applied in-place on XT (per-partition scale+shift).
  - leaky_relu(z) is decomposed as alpha*z + (1-alpha)*relu(z). The linear
    part is folded analytically into the first MLP layer; only
    R = sum_k relu(z_k + b_me) is computed at full resolution:
      * entity matmuls: K=32 zero-padded block weights, one entity-pair per
        matmul, 4 concurrent via tile_position row groups, fp32 PSUM
      * relu+bias: split across ScalarE (activation) and VectorE
        (tensor_scalar add+max), 3 PSUM banks per instruction, bf16 out
      * pooling over the 100 entities: PE matmul with a 0/1 selector,
        accumulated in PSUM
  - MLP: leaky layers via max identity — h = a*p + (1-a)*relu(p) computed as
    two accumulating matmuls on (p, relu(p)); softmax via PE transpose to
    batch-major then Exp + reciprocal (no max subtraction: logits are O(1)).
"""

import sys
import numpy as np

sys.path.insert(0, "/opt/trn_rl_repo")

import ml_dtypes

B_FULL, D, H, A = 16384, 204, 64, 32
NCORES = 8
BL = B_FULL // NCORES          # 2048 rows per core
NBT = 4                        # batch tiles per core
NT = BL // NBT                 # 512 columns per batch tile
K_ENT = 100                    # entities
NPAIR = 50                     # entity pairs (2 entities / matmul)
ALPHA = 0.01                   # jax.nn.leaky_relu default negative_slope
EPS = 1e-5
C = 256                        # padded feature columns (bf16 layout)
DPAD = 228                     # host-padded X width (features 204..227 = 0)

# --- column layout: block k of 32 sbuf columns <- features 28k .. 28k+31 ---
# (one uniform overlapping-window DMA per tile; the 4 duplicated columns per
# block are never selected by Wall/msel).  Pair p (features 4+4p..7+4p) lives
# in block k=(4+4p)//28 at column 4+4p+4k; pairs are 4-aligned and blocks
# start at multiples of 28 (also 4-aligned), so pairs never straddle blocks.
PAIR_COL = np.array([4 + 4 * p + 4 * ((4 + 4 * p) // 28) for p in range(NPAIR)])
for p in range(NPAIR):
    c = PAIR_COL[p]
    k = c // 32
    assert c % 4 == 0 and c % 32 <= 24 and 28 * k <= 4 + 4 * p <= 28 * k + 24


def _feat_of_col():
    f = np.full(C, -1, np.int64)
    for c in range(C):
        k, r = c // 32, c % 32
        if r < 28 and 28 * k + r < D:
            f[c] = 28 * k + r
    return f


FEAT_OF_COL = _feat_of_col()

PAIR_FILL = PAIR_COL // 128            # which transpose block (XT region)
PAIR_PART = PAIR_COL % 128             # partition of first row
PAIR_QUAD = PAIR_PART // 32            # row-group quadrant
PAIR_SLOT = (PAIR_PART % 32) // 4      # slot within quadrant (selects lhsT block)

# round-robin issue order across quadrants
_QLISTS = [[p for p in range(NPAIR) if PAIR_QUAD[p] == g] for g in range(4)]
PAIR_ORDER = []
for t in range(max(len(q) for q in _QLISTS)):
    for g in range(4):
        if t < len(_QLISTS[g]):
            PAIR_ORDER.append(_QLISTS[g][t])
assert len(PAIR_ORDER) == NPAIR

_prog_cache = {}


def _build_host_constants(W_me, b_me, W1, b1, W2, b2, W3, b3):
    bf16 = ml_dtypes.bfloat16
    # Wall [128, 8*128]: for quadrant row r (0..31) and slot m: rows 4m..4m+3
    # hold the entity-pair weight block, other rows zero.  Wall same for all
    # quadrants -> replicate pattern to all 128 partitions.
    pat = np.zeros((32, 8 * 128), np.float32)
    for m in range(8):
        for j in range(2):          # entity within pair
            for e in range(2):      # input dim
                # row 4m+2j+e, columns m*128 + (64j .. 64j+63) = W_me[:, e]
                pat[4 * m + 2 * j + e, m * 128 + 64 * j: m * 128 + 64 * (j + 1)] = W_me[:, e]
    Wall = np.tile(pat, (4, 1)).astype(bf16)

    sel = np.zeros((128, 64), np.float32)
    for j in range(2):
        sel[np.arange(64) + 64 * j, np.arange(64)] = 1.0
    selpack = np.concatenate([sel, sel], axis=1).astype(ml_dtypes.float8_e4m3)
    sel = sel.astype(bf16)

    # msel masks: [128, 32] per XT region (cols >=2 all-zero: they produce
    # zero rows 66..95 of pol_vec for free).  Only columns holding a pair
    # contribute: duplicated/pad columns are excluded.
    mselA = np.zeros((128, 32), np.float32)
    mselB = np.zeros((128, 32), np.float32)
    pair_cols = set()
    for p in range(NPAIR):
        for off in range(4):
            pair_cols.add(int(PAIR_COL[p]) + off)
    for c in range(C):
        f = FEAT_OF_COL[c]
        if c in pair_cols and f >= 4:
            (mselA if c < 128 else mselB)[c % 128, (f - 4) % 2] = 1.0
    mselA = mselA.astype(bf16)
    mselB = mselB.astype(bf16)

    bvec = np.tile(b_me, 2).reshape(128, 1).astype(np.float32)

    # first MLP layer folded weights: pol_vec rows 0..63 = R_raw,
    # rows 64..65 = m_raw, rows 96..99 = head (Xn[:, :4]), rest zero.
    W1h = W1[:, :4]
    W1b = W1[:, 4:]
    lhsT_h1 = np.zeros((128, 64), np.float32)
    lhsT_h1[0:64, :] = ((1.0 - ALPHA) / K_ENT) * W1b.T
    lhsT_h1[64:66, :] = (ALPHA / K_ENT) * (W1b @ W_me).T
    lhsT_h1[96:100, :] = W1h.T
    b1vec = (b1 + ALPHA * (W1b @ b_me)).reshape(64, 1).astype(np.float32)

    h2a = (ALPHA * W2).T.astype(np.float32).copy()
    h2b = ((1.0 - ALPHA) * W2).T.astype(np.float32).copy()
    b2vec = b2.reshape(64, 1).astype(np.float32)
    h3a = (ALPHA * W3).T.astype(np.float32).copy()
    h3b = ((1.0 - ALPHA) * W3).T.astype(np.float32).copy()
    b3vec = b3.reshape(32, 1).astype(np.float32)

    ident = np.eye(128, dtype=np.float32).astype(bf16)   # PE transpose identity (bf16)
    ident32 = np.eye(32, dtype=np.float32)               # logits transpose identity (f32)
    onesb = np.ones((128, 1), np.float32).astype(bf16)
    onesf = np.ones((128, 1), np.float32)

    return dict(Wall=Wall, sel=sel, selpack=selpack, mselA=mselA, mselB=mselB, bvec=bvec,
                lhsT_h1=lhsT_h1, b1vec=b1vec, h2a=h2a, h2b=h2b, b2vec=b2vec,
                h3a=h3a, h3b=h3b, b3vec=b3vec, ident=ident, ident32=ident32,
                onesb=onesb, onesf=onesf)


def build_program(num_devices=NCORES):
    """Emit the SPMD Bass program (identical on every core)."""
    from contextlib import ExitStack
    import concourse.bass as bass
    import concourse.bacc as bacc
    import concourse.tile as tile
    from concourse import mybir

    fp32 = mybir.dt.float32
    bf16 = mybir.dt.bfloat16
    fp8 = mybir.dt.float8e4
    ALU = mybir.AluOpType
    ACTF = mybir.ActivationFunctionType

    nc = bacc.Bacc(None, num_devices=num_devices)

    X = nc.declare_dram_parameter("X", [BL, DPAD], fp32, isOutput=False)
    OUT = nc.declare_dram_parameter("OUT", [BL, A], fp32, isOutput=True)
    dparams = {}
    for name, shape, dt in [
        ("Wall", [128, 1024], bf16), ("sel", [128, 64], bf16),
        ("selpack", [128, 128], fp8),
        ("mselA", [128, 32], bf16), ("mselB", [128, 32], bf16),
        ("bvec", [128, 1], fp32), ("lhsT_h1", [128, 64], fp32),
        ("b1vec", [64, 1], fp32), ("h2a", [64, 64], fp32), ("h2b", [64, 64], fp32),
        ("b2vec", [64, 1], fp32), ("h3a", [64, 32], fp32), ("h3b", [64, 32], fp32),
        ("b3vec", [32, 1], fp32), ("ident", [128, 128], bf16),
        ("ident32", [32, 32], fp32), ("onesb", [128, 1], bf16),
        ("onesf", [128, 1], fp32),
    ]:
        dparams[name] = nc.declare_dram_parameter(name, shape, dt, isOutput=False)

    with tile.TileContext(nc) as tc, ExitStack() as ctx:
        singles = ctx.enter_context(tc.tile_pool(name="singles", bufs=1))
        xtp = ctx.enter_context(tc.tile_pool(name="xtp", bufs=1))

        cst = {}
        for name, p in dparams.items():
            t = singles.tile(list(p.shape), p.dtype, tag=f"cst_{name}")
            nc.sync.dma_start(out=t[:], in_=p[:])
            cst[name] = t

        # XT: feature-major bf16, region F at cols F*BL .. F*BL+BL
        xt = xtp.tile([128, 2 * BL], bf16)
        xt3 = xt.rearrange("p (s n) -> p s n", s=2)

        # ---------------- pre-phase: load, stats, transpose ----------------
        with ExitStack() as pre:
            nat = pre.enter_context(tc.tile_pool(name="nat", bufs=6))
            f32p = pre.enter_context(tc.tile_pool(name="f32p", bufs=4))
            sqp = pre.enter_context(tc.tile_pool(name="sqp", bufs=3))
            pps = pre.enter_context(tc.tile_pool(name="pps", bufs=2, space="PSUM"))
            sps = pre.enter_context(tc.tile_pool(name="sps", bufs=2, space="PSUM"))
            stp = pre.enter_context(tc.tile_pool(name="stp", bufs=4))

            ps_sx = sps.tile([1, C], fp32, tag="psx")
            ps_sq = sps.tile([1, C], fp32, tag="psq")

            ntile = BL // 128
            for i in range(ntile):
                xf = f32p.tile([128, DPAD], fp32)
                nc.sync.dma_start(out=xf[:], in_=X[128 * i:128 * (i + 1), :])
                # cast + overlapping-window expand: block k of 32 cols <-
                # features 28k..28k+31 (f32 -> bf16)
                xb = nat.tile([128, C], bf16)
                win = bass.AP(
                    tensor=xf.tensor, offset=xf.offset,
                    ap=[list(xf.ap[0]), [28, 8], [1, 32]],
                )
                nc.vector.tensor_copy(xb[:], win)
                sq = sqp.tile([128, C], fp32)
                nc.vector.tensor_tensor(out=sq[:], in0=xb[:], in1=xb[:], op=ALU.mult)
                nc.tensor.matmul(ps_sx[:], cst["onesb"][:], xb[:],
                                 start=(i == 0), stop=(i == ntile - 1))
                nc.tensor.matmul(ps_sq[:], cst["onesf"][:], sq[:],
                                 start=(i == 0), stop=(i == ntile - 1))
                # transpose both 128-col blocks -> bf16 PSUM -> XT
                pt = pps.tile([128, C], bf16)
                nc.tensor.transpose(pt[:, 0:128], xb[:, 0:128], cst["ident"][:])
                nc.tensor.transpose(pt[:, 128:256], xb[:, 128:256], cst["ident"][:])
                pt3 = pt.rearrange("p (s n) -> p s n", s=2)
                nc.vector.tensor_copy(xt3[:, :, 128 * i:128 * (i + 1)], pt3[:, :, :])

            # --- stats: evacuate sums, AllReduce, reshape per-partition ---
            st_sb = stp.tile([1, 2 * C], fp32)
            nc.scalar.activation(st_sb[:, 0:C], ps_sx[:], ACTF.Copy)
            nc.scalar.activation(st_sb[:, C:2 * C], ps_sq[:], ACTF.Copy)
            cc_in = nc.dram_tensor("cc_in", [1, 2 * C], fp32)
            cc_out = nc.dram_tensor("cc_out", [1, 2 * C], fp32, addr_space="Shared")
            nc.sync.dma_start(out=cc_in[:], in_=st_sb[:])
            nc.gpsimd.collective_compute(
                "AllReduce", ALU.add,
                replica_groups=[list(range(num_devices))],
                ins=[cc_in[:]], outs=[cc_out[:]],
            )
            # view: element (p2, e) = flat[e*2C... e*256... e*C? sums at 0..C-1, sumsq at C..2C-1
            cc_v = cc_out[:].rearrange("a (e p) -> (a p) e", e=2)
            # per-partition stats for both regions at once: st[p, F] layout
            st = stp.tile([128, 2, 2], fp32, tag="st")   # [p, F, (sx, sq)]
            for F in range(2):
                nc.sync.dma_start(out=st[:, F, :], in_=cc_v[128 * F:128 * (F + 1), :])
            stf = st.rearrange("p f e -> p (f e)")
            mus = stp.tile([128, 2], fp32, tag="mus")    # mu per region
            ex2 = stp.tile([128, 2], fp32, tag="ex2")
            nc.vector.tensor_scalar(out=mus[:], in0=stf[:, 0:4:2], scalar1=1.0 / B_FULL,
                                    scalar2=None, op0=ALU.mult)
            nc.vector.tensor_scalar(out=ex2[:], in0=stf[:, 1:4:2], scalar1=1.0 / B_FULL,
                                    scalar2=None, op0=ALU.mult)
            mu2 = stp.tile([128, 2], fp32, tag="mu2")
            nc.vector.tensor_tensor(out=mu2[:], in0=mus[:], in1=mus[:], op=ALU.mult)
            vpe = stp.tile([128, 2], fp32, tag="vpe")
            nc.vector.tensor_tensor(out=vpe[:], in0=ex2[:], in1=mu2[:], op=ALU.subtract)
            nc.vector.tensor_scalar(out=vpe[:], in0=vpe[:], scalar1=EPS, scalar2=None,
                                    op0=ALU.add)
            rs = stp.tile([128, 2], fp32, tag="rs")
            nc.vector.reciprocal(rs[:], vpe[:])          # 1/(var+eps)
            s0 = stp.tile([128, 2], fp32, tag="s0")
            nc.scalar.activation(s0[:], rs[:], ACTF.Sqrt)
            # one Newton step for sqrt: s = s0*(1.5 - 0.5*vpe*s0^2); vpe = 1/rs
            u = stp.tile([128, 2], fp32, tag="u")
            nc.vector.tensor_tensor(out=u[:], in0=s0[:], in1=s0[:], op=ALU.mult)
            nc.vector.tensor_tensor(out=u[:], in0=u[:], in1=vpe[:], op=ALU.mult)
            nc.vector.tensor_scalar(out=u[:], in0=u[:], scalar1=-0.5, scalar2=1.5,
                                    op0=ALU.mult, op1=ALU.add)
            svec = stp.tile([128, 2], fp32, tag="sv")
            nc.vector.tensor_tensor(out=svec[:], in0=s0[:], in1=u[:], op=ALU.mult)
            nmvec = stp.tile([128, 2], fp32, tag="nm")
            nc.vector.tensor_tensor(out=nmvec[:], in0=mus[:], in1=svec[:], op=ALU.mult)
            nc.vector.tensor_scalar(out=nmvec[:], in0=nmvec[:], scalar1=-1.0,
                                    scalar2=None, op0=ALU.mult)
            for F in range(2):
                nc.vector.tensor_scalar(
                    out=xt[:, BL * F:BL * (F + 1)], in0=xt[:, BL * F:BL * (F + 1)],
                    scalar1=svec[:, F:F + 1], scalar2=nmvec[:, F:F + 1],
                    op0=ALU.mult, op1=ALU.add)

        # ---------------- main phase ----------------
        zpsp = ctx.enter_context(tc.tile_pool(name="zpsp", bufs=1, space="PSUM"))
        mlpp = ctx.enter_context(tc.tile_pool(name="mlpp", bufs=2, space="PSUM"))
        ypool = ctx.enter_context(tc.tile_pool(name="ypool", bufs=10))
        polp = ctx.enter_context(tc.tile_pool(name="polp", bufs=3))
        mlps = ctx.enter_context(tc.tile_pool(name="mlps", bufs=4))
        outp = ctx.enter_context(tc.tile_pool(name="outp", bufs=4))

        zps = zpsp.tile([128, 6 * 512], fp32)

        # HAM warmup: ~4us of dense back-to-back matmuls reading the
        # just-normalized xt so they execute right before the main loop
        # (the PE clock only unthrottles after a contiguous busy window;
        # the pipelined main loop alone never provides one).  Uses a real
        # accumulation group into a scratch PSUM tile; results unused.
        warm = mlpp.tile([32, NT], fp32, tag="mlp")
        for w in range(10):
            nc.tensor.matmul(warm[:], cst["mselA"][:], xt[:, 0:NT],
                             start=(w == 0), stop=(w == 9))

        for bt in range(NBT):
            col0 = bt * NT
            acc = mlpp.tile([64, NT], fp32, tag="mlp")
            groups = [PAIR_ORDER[i:i + 3] for i in range(0, NPAIR, 3)]
            ngrp = len(groups)

            def emit_z(gi):
                base = (gi * 3) % 6
                for j, p in enumerate(groups[gi]):
                    g = PAIR_QUAD[p]
                    m = PAIR_SLOT[p]
                    F = PAIR_FILL[p]
                    nc.tensor.matmul(
                        zps[:, (base + j) * 512:(base + j + 1) * 512],
                        cst["Wall"][32 * g:32 * (g + 1), 128 * m:128 * (m + 1)],
                        xt[32 * g:32 * (g + 1), BL * F + col0:BL * F + col0 + NT],
                        start=True, stop=True,
                        tile_position=(32 * int(g), 0),
                    )

            ytiles = {}

            def emit_relu(gi):
                base = (gi * 3) % 6
                grp = groups[gi]
                n = len(grp)
                y = ypool.tile([128, 3 * 512], bf16)
                ytiles[gi] = y
                zv = zps[:, base * 512:(base + n) * 512]
                if gi % 2 == 0:
                    nc.scalar.activation(y[:, 0:n * 512], zv, ACTF.Relu,
                                         bias=cst["bvec"][:], scale=1.0)
                else:
                    nc.vector.tensor_scalar(out=y[:, 0:n * 512], in0=zv,
                                            scalar1=cst["bvec"][:], scalar2=0.0,
                                            op0=ALU.add, op1=ALU.max)

            def emit_pool(gi):
                y = ytiles.pop(gi)
                for j, p in enumerate(groups[gi]):
                    idx = gi * 3 + j
                    nc.tensor.matmul(acc[:], cst["sel"][:], y[:, j * 512:(j + 1) * 512],
                                     start=(idx == 0), stop=(idx == NPAIR - 1))

            # software pipeline: z-matmuls 2 groups ahead of relu (6 PSUM
            # banks = 2 groups of 3 in flight; relu(gi-2) must precede z(gi)
            # in program order since they share slices), pool lags 2 more.
            for gi in range(ngrp + 4):
                if gi >= 4:
                    emit_pool(gi - 4)
                if 2 <= gi < ngrp + 2:
                    emit_relu(gi - 2)
                if gi < ngrp:
                    emit_z(gi)

            # ---- MLP tail ----
            # pol rows: 0..63 = R_raw, 64..65 = m_raw (66..95 zero via msel
            # zero columns), 96..99 = head (100..127 junk; lhsT_h1 rows are 0)
            pol = polp.tile([128, NT], fp32)
            nc.scalar.activation(pol[0:64, :], acc[:], ACTF.Copy)       # R_raw
            ps_m = mlpp.tile([32, NT], fp32, tag="mlp")
            nc.tensor.matmul(ps_m[:], cst["mselA"][:], xt[:, col0:col0 + NT],
                             start=True, stop=False)
            nc.tensor.matmul(ps_m[:], cst["mselB"][:], xt[:, BL + col0:BL + col0 + NT],
                             start=False, stop=True)
            nc.scalar.activation(pol[64:96, :], ps_m[:], ACTF.Copy)     # m_raw + zeros
            nc.scalar.activation(pol[96:128, :], xt[0:32, col0:col0 + NT], ACTF.Copy)

            ps_h1 = mlpp.tile([64, NT], fp32, tag="mlp")
            nc.tensor.matmul(ps_h1[:], cst["lhsT_h1"][:], pol[:], start=True, stop=True)
            p1 = mlps.tile([64, NT], fp32, tag="p")
            r1 = mlps.tile([64, NT], fp32, tag="r")
            nc.vector.tensor_scalar(out=p1[:], in0=ps_h1[:], scalar1=cst["b1vec"][:],
                                    scalar2=None, op0=ALU.add)
            nc.scalar.activation(r1[:], ps_h1[:], ACTF.Relu, bias=cst["b1vec"][:])

            ps_h2 = mlpp.tile([64, NT], fp32, tag="mlp")
            nc.tensor.matmul(ps_h2[:], cst["h2a"][:], p1[:], start=True, stop=False)
            nc.tensor.matmul(ps_h2[:], cst["h2b"][:], r1[:], start=False, stop=True)
            p2 = mlps.tile([64, NT], fp32, tag="p")
            r2 = mlps.tile([64, NT], fp32, tag="r")
            nc.vector.tensor_scalar(out=p2[:], in0=ps_h2[:], scalar1=cst["b2vec"][:],
                                    scalar2=None, op0=ALU.add)
            nc.scalar.activation(r2[:], ps_h2[:], ACTF.Relu, bias=cst["b2vec"][:])

            ps_lg = mlpp.tile([32, NT], fp32, tag="mlp")
            nc.tensor.matmul(ps_lg[:], cst["h3a"][:], p2[:], start=True, stop=False)
            nc.tensor.matmul(ps_lg[:], cst["h3b"][:], r2[:], start=False, stop=True)
            lg = mlps.tile([32, NT], fp32, tag="lg")
            nc.scalar.activation(lg[:], ps_lg[:], ACTF.Identity, bias=cst["b3vec"][:])

            # ---- softmax over A=32 (transpose to batch-major) ----
            ps_tr = mlpp.tile([128, 128], fp32, tag="mlp")
            for s in range(4):
                nc.tensor.transpose(ps_tr[:, 32 * s:32 * (s + 1)],
                                    lg[:, 128 * s:128 * (s + 1)], cst["ident32"][:])
            esb = outp.tile([128, 128], fp32, tag="e")
            nc.scalar.activation(esb[:], ps_tr[:], ACTF.Exp)
            e3 = esb.rearrange("p (s a) -> p s a", s=4)
            sums = outp.tile([128, 4], fp32, tag="sums")
            nc.vector.tensor_reduce(out=sums[:], in_=e3[:, :, :],
                                    axis=mybir.AxisListType.X, op=ALU.add)
            rec = outp.tile([128, 4], fp32, tag="rec")
            nc.vector.reciprocal(rec[:], sums[:])
            fin = outp.tile([128, 128], fp32, tag="fin")
            fin3 = fin.rearrange("p (s a) -> p s a", s=4)
            rec_b = rec[:].unsqueeze(2).broadcast_to([128, 4, 32])
            nc.vector.tensor_tensor(out=fin3[:, :, :], in0=e3[:, :, :], in1=rec_b,
                                    op=ALU.mult)
            for s in range(4):
                nc.sync.dma_start(
                    out=OUT[col0 + 128 * s: col0 + 128 * (s + 1), :],
                    in_=fin3[:, s, :],
                )
    nc.finalize()
    return nc


def kernel(**inputs):
    X = np.asarray(inputs["X"], np.float32)
    consts = _build_host_constants(
        np.asarray(inputs["W_me"], np.float32), np.asarray(inputs["b_me"], np.float32),
        np.asarray(inputs["W1"], np.float32), np.asarray(inputs["b1"], np.float32),
        np.asarray(inputs["W2"], np.float32), np.asarray(inputs["b2"], np.float32),
        np.asarray(inputs["W3"], np.float32), np.asarray(inputs["b3"], np.float32),
    )
    from concourse.bass_utils import run_bass_kernel_spmd

    if "nc" not in _prog_cache:
        _prog_cache["nc"] = build_program(NCORES)
    nc = _prog_cache["nc"]

    Xpad = np.zeros((B_FULL, DPAD), np.float32)
    Xpad[:, :D] = X
    in_maps = []
    for i in range(NCORES):
        m = {"X": np.ascontiguousarray(Xpad[i * BL:(i + 1) * BL])}
        m.update(consts)
        in_maps.append(m)
    res = run_bass_kernel_spmd(nc, in_maps, list(range(NCORES)))
    out = np.concatenate([res.results[i]["OUT"] for i in range(NCORES)], axis=0)
    return out.astype(np.float32)
